# revision 3
# baseline (speedup 1.0000x reference)
"""Trainium2 Bass kernel for nn_ChimeraNet (encoder -> 10-step Euler RNN -> LN -> readout).

Data-parallel over 8 NeuronCores: each core gets 1024 rows of the batch and a
replicated set of (host-prefolded) weights.

Math (per core, R=1024 rows, D=1024), with the state scaled u = h/0.2 so the
update is one fused op (0.2 folded into W_res and the readout on the host):
    drive_in = x @ W_c + bias          with W_c = W_enc.T @ W_in (host-folded)
    u' = 0.8 u + tanh(u @ (0.2 W_res) + drive_in),  u_0 = 0, 10 steps
    out = inv*(h @ W2.T) + (-mu*inv)*w1 + b2       (LayerNorm folded, h = 0.2 u)

fp8 fast path: the 9 recurrence matmuls run as fp8e4m3 DoubleRow (2x PE rate,
K=256 per instruction).  The matmul operand is u8 = fp8(u); W_res is scaled by
S=128 before fp8 quantization to clear the subnormal range, and the 1/S is
applied in the drive STT.  The state u is kept in bf16 (updated from the
unquantized bf16 tanh, which roughly halves the error vs fp8-tau feedback),
and all dtype conversions run on cast-DMAs (compute-engine casts take a
microcoded slow path).  Per step per (m-pair, n-half) group:
    PE   : 8 DoubleRow matmuls -> 2-bank PSUM tile      (u8 @ W8)
    DVE  : d = P*(1/S) + drive_in      (STT, flat [128,1024] APs)
    Act  : tau = tanh(d)               (f32->f32)
    DMA  : tau_b = bf16(tau); after the u STT, u8 = fp8(u)
    DVE  : u = 0.8 u + tau_b           (bf16, 2x DVE mode)
Encoder runs in bf16 (x pre-transposed/padded to [896, R] on the host - a
layout-only transform); LN stats via PE ones/readout-column tricks as before.
"""

import os
import sys

import numpy as np

try:
    import concourse.bass as bass  # noqa: F401
except ImportError:  # pragma: no cover - fresh grading env without PYTHONPATH
    for p in ("/root/.axon_site", "/root/.axon_site/_ro/trn_rl_repo",
              "/root/.axon_site/_ro/pypackages", "/opt/trn_rl_repo"):
        if os.path.isdir(p) and p not in sys.path:
            sys.path.append(p)
    import concourse.bass as bass

from contextlib import ExitStack

import ml_dtypes
import concourse.tile as tile
from concourse import bacc, bass_utils, mybir
from concourse.masks import make_identity

N_CORES = 8
B = 8192
R = B // N_CORES        # rows per core
D = 1024                # latent dim
KX = 784                # encoder input dim
KXP = 896               # padded to 7*128
DT_STEP = 0.2
STEPS = 10
EPS = 1e-5
S_W = 128.0             # fp8 scale on W_res

F32 = mybir.dt.float32
F32R = mybir.dt.float32r
BF16 = mybir.dt.bfloat16
FP8 = mybir.dt.float8e4
AF = mybir.ActivationFunctionType
ALU = mybir.AluOpType
DR = mybir.MatmulPerfMode.DoubleRow

KD = D // 128            # 8 k/m tiles over D
KE = KXP // 128          # 7 encoder k tiles
NS = 2                   # row halves of 512
NWARM = 8


def _build_program():
    nc = bacc.Bacc("TRN2", target_bir_lowering=False, debug=False)

    xt = nc.dram_tensor("xt", [KXP, R], BF16, kind="ExternalInput").ap()
    wc = nc.dram_tensor("wc", [KXP, D], BF16, kind="ExternalInput").ap()
    w8 = nc.dram_tensor("w8", [D, D], FP8, kind="ExternalInput").ap()
    bias = nc.dram_tensor("bias", [D], F32, kind="ExternalInput").ap()
    w2a = nc.dram_tensor("w2a", [D, 11], BF16, kind="ExternalInput").ap()
    w1 = nc.dram_tensor("w1", [10], F32, kind="ExternalInput").ap()
    b2 = nc.dram_tensor("b2", [10], F32, kind="ExternalInput").ap()
    out = nc.dram_tensor("out", [R, 10], F32, kind="ExternalOutput").ap()

    with tile.TileContext(nc) as tc, ExitStack() as ctx:
        state = ctx.enter_context(tc.tile_pool(name="state", bufs=1))
        consts = ctx.enter_context(tc.tile_pool(name="consts", bufs=1))
        trans = ctx.enter_context(tc.tile_pool(name="trans", bufs=3))

        # persistent SBUF state, n-major so group ops are flat [128, 1024]
        drive = [state.tile([128, KD, 512], F32, name=f"dr{n}", tag=f"dr{n}")
                 for n in range(NS)]
        u_sb = [state.tile([128, KD, 512], BF16, name=f"u{n}", tag=f"u{n}")
                for n in range(NS)]
        u8_sb = [state.tile([128, KD, 512], FP8, name=f"u8{n}", tag=f"u8{n}")
                 for n in range(NS)]
        sq_sb = [state.tile([128, KD, 512], BF16, name=f"sq{n}", tag=f"sq{n}")
                 for n in range(NS)]
        w8_sb = state.tile([128, KD, D], FP8, name="w8sb")
        wc_sb = state.tile([128, KE, D], BF16, name="wcsb")
        xt_sb = state.tile([128, KE, R], BF16, name="xtsb")

        # input DMAs: x + bias on the sync queue, weights on the scalar queue
        nc.sync.dma_start(out=xt_sb, in_=xt.rearrange("(k p) r -> p k r", p=128))
        bias_sb = consts.tile([128, KD], F32)
        nc.sync.dma_start(out=bias_sb, in_=bias.rearrange("(m p) -> p m", p=128))
        nc.scalar.dma_start(out=wc_sb, in_=wc.rearrange("(k p) c -> p k c", p=128))
        nc.scalar.dma_start(out=w8_sb, in_=w8.rearrange("(k p) c -> p k c", p=128))

        ident = consts.tile([128, 128], F32)
        make_identity(nc, ident)
        w2a_sb = consts.tile([128, KD, 11], BF16)
        nc.scalar.dma_start(out=w2a_sb, in_=w2a.rearrange("(k p) o -> p k o", p=128))
        ones_bf = consts.tile([128, 1], BF16)
        nc.vector.memset(ones_bf, 1.0)
        eps_sb = consts.tile([128, 1], F32)
        nc.vector.memset(eps_sb, EPS)
        w1_bc = consts.tile([128, 10], F32)
        nc.scalar.dma_start(out=w1_bc, in_=bass.AP(tensor=w1.tensor, offset=w1.offset,
                                                   ap=[[0, 128]] + list(w1.ap)))
        b2_bc = consts.tile([128, 10], F32)
        nc.scalar.dma_start(out=b2_bc, in_=bass.AP(tensor=b2.tensor, offset=b2.offset,
                                                   ap=[[0, 128]] + list(b2.ap)))

        with ExitStack() as mmctx:
            encctx = ExitStack()
            psum = encctx.enter_context(
                tc.tile_pool(name="mm", bufs=4, space="PSUM"))

            # PE warmup: dependency-free matmuls at t~0 keep the HAM clock
            # gate at 8/8 while the input DMAs are in flight.
            warm_src = consts.tile([128, 256], F32)
            nc.vector.memset(warm_src, 0.01)
            warm_sb = consts.tile([128, 1], F32)
            for w in range(NWARM):
                wp = psum.tile([128, 512], F32, name=f"warm{w}", tag="mm")
                nc.tensor.matmul(wp[:, :256], lhsT=warm_src[:, :128], rhs=warm_src,
                                 start=True, stop=True)
                if w == NWARM - 1:
                    nc.vector.tensor_copy(warm_sb, wp[:, :1])  # keep-alive

            # ---------------- encoder: drive = (x @ W_c + bias).T ----------
            for n in range(NS):
                sl = slice(n * 512, (n + 1) * 512)
                for m in range(KD):
                    ps = psum.tile([128, 512], F32, name=f"eps{n}_{m}", tag="mm")
                    for k in range(KE):
                        nc.tensor.matmul(
                            ps, lhsT=wc_sb[:, k, m * 128:(m + 1) * 128],
                            rhs=xt_sb[:, k, sl],
                            start=(k == 0), stop=(k == KE - 1))
                    nc.scalar.activation(drive[n][:, m, :], ps, AF.Identity,
                                         bias=bias_sb[:, m:m + 1], scale=1.0)

            # ---------------- step 0: u1 = tanh(drive_in) ------------------
            for n in range(NS):
                for g in range(KD // 2):
                    t0 = trans.tile([128, 1024], F32, name=f"t0_{n}_{g}", tag="tau")
                    nc.scalar.activation(t0, drive[n][:, 2 * g:2 * g + 2, :].opt(),
                                         AF.Tanh)
                    nc.gpsimd.dma_start(out=u_sb[n][:, 2 * g:2 * g + 2, :], in_=t0)
                    nc.gpsimd.dma_start(out=u8_sb[n][:, 2 * g:2 * g + 2, :], in_=t0)

            # ---------------- Euler loop, steps 1..9 -----------------------
            encctx.close()
            loopctx = ExitStack()
            psum2 = loopctx.enter_context(
                tc.tile_pool(name="mm2", bufs=4, space="PSUM"))

            for s in range(1, STEPS):
                last = s == STEPS - 1
                for n in range(NS):
                    for g in range(KD // 2):
                        ps = psum2.tile([128, 2, 512], F32, name=f"p{s}_{n}_{g}",
                                        tag="mm2")
                        for kp in range(KD // 2):
                            for mm in range(2):
                                mcol = (2 * g + mm) * 128
                                nc.tensor.matmul(
                                    ps[:, mm, :],
                                    lhsT=w8_sb[:, 2 * kp:2 * kp + 2,
                                               mcol:mcol + 128],
                                    rhs=u8_sb[n][:, 2 * kp:2 * kp + 2, :],
                                    perf_mode=DR,
                                    start=(kp == 0), stop=(kp == KD // 2 - 1))
                        d = trans.tile([128, 1024], F32, name=f"d{s}_{n}_{g}",
                                       tag="d")
                        nc.vector.scalar_tensor_tensor(
                            d, in0=ps.opt(), scalar=1.0 / S_W,
                            in1=drive[n][:, 2 * g:2 * g + 2, :].opt(),
                            op0=ALU.mult, op1=ALU.add)
                        t = trans.tile([128, 1024], F32, name=f"t{s}_{n}_{g}",
                                       tag="tau")
                        nc.scalar.activation(t, d, AF.Tanh)
                        tb = trans.tile([128, 1024], BF16, name=f"tb{s}_{n}_{g}",
                                        tag="tb")
                        nc.gpsimd.dma_start(out=tb, in_=t)
                        ug = u_sb[n][:, 2 * g:2 * g + 2, :].opt()
                        nc.vector.scalar_tensor_tensor(
                            ug, in0=ug, scalar=1.0 - DT_STEP, in1=tb,
                            op0=ALU.mult, op1=ALU.add)
                        if not last:
                            nc.gpsimd.dma_start(
                                out=u8_sb[n][:, 2 * g:2 * g + 2, :],
                                in_=u_sb[n][:, 2 * g:2 * g + 2, :])
                        else:
                            sg = sq_sb[n][:, 2 * g:2 * g + 2, :].opt()
                            nc.vector.tensor_tensor(sg, ug, ug, ALU.mult)

            # ---------------- tail: readout + LN stats ---------------------
            loopctx.close()
            psum = mmctx.enter_context(
                tc.tile_pool(name="mmt", bufs=2, space="PSUM"))
            tail = ctx.enter_context(tc.tile_pool(name="tail", bufs=1))
            s2_sb = tail.tile([1, R], F32)
            y_sb = tail.tile([11, R], F32)

            tp2ctx = ExitStack()
            tp2 = tp2ctx.enter_context(
                tc.tile_pool(name="tp2", bufs=4, space="PSUM"))
            for n in range(NS):
                sl = slice(n * 512, (n + 1) * 512)
                yp = psum.tile([11, 512], F32, name=f"yp{n}", tag="mm")
                for k in range(KD):
                    nc.tensor.matmul(yp, lhsT=w2a_sb[:, k, :],
                                     rhs=u_sb[n][:, k, :],
                                     start=(k == 0), stop=(k == KD - 1))
                nc.scalar.copy(y_sb[:, sl], yp)
                s2 = psum.tile([1, 512], F32, name=f"s2p{n}", tag="mm")
                for k in range(KD):
                    nc.tensor.matmul(s2, lhsT=ones_bf, rhs=sq_sb[n][:, k, :],
                                     start=(k == 0), stop=(k == KD - 1))
                nc.scalar.copy(s2_sb[:, sl], s2)

                for rt in range(n * 4, (n + 1) * 4):
                    rsl = slice(rt * 128, (rt + 1) * 128)
                    yn = tp2.tile([128, 11], F32, name=f"yn{rt}", tag="st")
                    nc.tensor.transpose(yn, y_sb[:, rsl], ident[:11, :11])
                    p2 = tp2.tile([128, 1], F32, name=f"p2_{rt}", tag="st")
                    nc.tensor.transpose(p2, s2_sb[:, rsl], ident[:1, :1])
                    mu_n = tail.tile([128, 1], F32, name=f"mu{rt}", tag="mu", bufs=2)
                    nc.scalar.mul(mu_n, yn[:, 10:11], -DT_STEP / D)   # -mean(h)
                    ex2 = tail.tile([128, 1], F32, name=f"ex2_{rt}", tag="ex2", bufs=2)
                    nc.scalar.mul(ex2, p2, DT_STEP * DT_STEP / D)     # E[h^2]
                    var = tail.tile([128, 1], F32, name=f"var{rt}", tag="var", bufs=2)
                    nc.vector.scalar_tensor_tensor(var, in0=mu_n, scalar=-1.0,
                                                   op0=ALU.mult, in1=mu_n,
                                                   op1=ALU.mult)
                    nc.vector.tensor_add(var, var, ex2)
                    sd = tail.tile([128, 1], F32, name=f"sd{rt}", tag="sd", bufs=2)
                    nc.scalar.activation(sd, var, AF.Sqrt, bias=eps_sb, scale=1.0)
                    inv = tail.tile([128, 1], F32, name=f"inv{rt}", tag="inv", bufs=2)
                    nc.vector.reciprocal(inv, sd)
                    qn = tail.tile([128, 1], F32, name=f"qn{rt}", tag="qn", bufs=2)
                    nc.vector.tensor_mul(qn, mu_n, inv)               # -mu*inv
                    t1 = tail.tile([128, 10], F32, name=f"t1_{rt}", tag="t1", bufs=2)
                    nc.vector.tensor_scalar_mul(t1, yn[:, 0:10], inv)
                    t2 = tail.tile([128, 10], F32, name=f"t2_{rt}", tag="t2", bufs=2)
                    nc.vector.scalar_tensor_tensor(t2, in0=w1_bc, scalar=qn,
                                                   in1=t1, op0=ALU.mult, op1=ALU.add)
                    o = tail.tile([128, 10], F32, name=f"o{rt}", tag="o", bufs=2)
                    nc.vector.tensor_add(o, t2, b2_bc)
                    nc.sync.dma_start(out=out[rsl, :], in_=o)
            tp2ctx.close()

    nc.compile()
    return nc


_NC_CACHE = None


def _get_program():
    global _NC_CACHE
    if _NC_CACHE is None:
        _NC_CACHE = _build_program()
    return _NC_CACHE


def _prepare_in_maps(inputs):
    x = np.asarray(inputs["x"], dtype=np.float32)
    w_enc = np.asarray(inputs["W_enc"], dtype=np.float32)
    w_res = np.asarray(inputs["W_res"], dtype=np.float32)
    w_in = np.asarray(inputs["W_in"], dtype=np.float32)
    bias = np.asarray(inputs["bias"], dtype=np.float32)
    ln_g = np.asarray(inputs["ln_g"], dtype=np.float32)
    ln_b = np.asarray(inputs["ln_b"], dtype=np.float32)
    w_out = np.asarray(inputs["W_out"], dtype=np.float32)
    b_out = np.asarray(inputs["b_out"], dtype=np.float32)

    w_c = (w_enc.T.astype(np.float64) @ w_in.astype(np.float64)).astype(np.float32)
    wc_pad = np.zeros((KXP, D), np.float32)
    wc_pad[:KX] = w_c
    w2 = w_out * ln_g[None, :]                       # [10, D]
    w8 = np.asarray(DT_STEP * S_W * w_res, dtype=ml_dtypes.float8_e4m3fn)
    w2a = np.zeros((D, 11), np.float32)
    w2a[:, :10] = DT_STEP * w2.T                     # readout: gives W2 @ h.T
    w2a[:, 10] = 1.0                                 # S1 column: sum_D u
    w1v = w2.sum(axis=1).astype(np.float32)
    b2v = (w_out.astype(np.float64) @ ln_b.astype(np.float64)
           + b_out.astype(np.float64)).astype(np.float32)

    shared = {
        "wc": np.ascontiguousarray(wc_pad.astype(ml_dtypes.bfloat16)),
        "w8": np.ascontiguousarray(w8),
        "bias": np.ascontiguousarray(bias),
        "w2a": np.ascontiguousarray(w2a.astype(ml_dtypes.bfloat16)),
        "w1": np.ascontiguousarray(w1v),
        "b2": np.ascontiguousarray(b2v),
    }
    in_maps = []
    for c in range(N_CORES):
        m = dict(shared)
        xt = np.zeros((KXP, R), np.float32)
        xt[:KX] = x[c * R:(c + 1) * R, :].T
        m["xt"] = np.ascontiguousarray(xt.astype(ml_dtypes.bfloat16))
        in_maps.append(m)
    return in_maps


def run(inputs, trace=False, tmpdir=None):
    """Run on 8 NeuronCores; returns (out [8192,10], BassKernelResults)."""
    nc = _get_program()
    in_maps = _prepare_in_maps(inputs)
    res = bass_utils.run_bass_kernel_spmd(
        nc, in_maps, core_ids=list(range(N_CORES)), trace=trace, tmpdir=tmpdir)
    outs = [np.asarray(r["out"]) for r in res.results]
    return np.concatenate(outs, axis=0), res


def kernel(**inputs):
    out, _ = run(inputs, trace=False)
    return out


# revision 6
# speedup vs baseline: 1.2488x; 1.2488x over previous
"""Trainium2 Bass kernel for nn_ChimeraNet (encoder -> 10-step Euler RNN -> LN -> readout).

Data-parallel over 8 NeuronCores: each core gets 1024 rows of the batch and a
replicated set of (host-prefolded) weights.

Math (per core, R=1024 rows, D=1024), with the state scaled u = h/0.2 so the
update is one fused op (0.2 folded into W_res and the readout on the host):
    drive_in = x @ W_c + bias          with W_c = W_enc.T @ W_in (host-folded)
    u' = 0.8 u + tanh(u @ (0.2 W_res) + drive_in),  u_0 = 0, 10 steps
    out = inv*(h @ W2.T) + (-mu*inv)*w1 + b2       (LayerNorm folded, h = 0.2 u)

fp8 fast path: the 9 recurrence matmuls run as fp8e4m3 DoubleRow (2x PE rate,
K=256 per instruction).  The matmul operand is u8 = fp8(u); W_res is scaled by
S=128 before fp8 quantization to clear the subnormal range, and the 1/S is
applied in the drive STT.  The state u is kept in bf16 (updated from the
unquantized bf16 tanh, which roughly halves the error vs fp8-tau feedback),
and all dtype conversions run on cast-DMAs (compute-engine casts take a
microcoded slow path).  Per step per (m-pair, n-half) group:
    PE   : 8 DoubleRow matmuls -> 2-bank PSUM tile      (u8 @ W8)
    DVE  : d = P*(1/S) + drive_in      (STT, flat [128,1024] APs)
    Act  : tau = tanh(d)               (f32->f32)
    DMA  : tau_b = bf16(tau); after the u STT, u8 = fp8(u)
    DVE  : u = 0.8 u + tau_b           (bf16, 2x DVE mode)
Encoder runs in bf16 (x pre-transposed/padded to [896, R] on the host - a
layout-only transform); LN stats via PE ones/readout-column tricks as before.
"""

import os
import sys

import numpy as np

try:
    import concourse.bass as bass  # noqa: F401
except ImportError:  # pragma: no cover - fresh grading env without PYTHONPATH
    for p in ("/root/.axon_site", "/root/.axon_site/_ro/trn_rl_repo",
              "/root/.axon_site/_ro/pypackages", "/opt/trn_rl_repo"):
        if os.path.isdir(p) and p not in sys.path:
            sys.path.append(p)
    import concourse.bass as bass

from contextlib import ExitStack

import ml_dtypes
import concourse.tile as tile
from concourse import bacc, bass_utils, mybir
from concourse.masks import make_identity

N_CORES = 8
B = 8192
R = B // N_CORES        # rows per core
D = 1024                # latent dim
KX = 784                # encoder input dim
KXP = 896               # padded to 7*128
DT_STEP = 0.2
STEPS = 10
EPS = 1e-5
S_W = 128.0             # fp8 scale on W_res

F32 = mybir.dt.float32
F32R = mybir.dt.float32r
BF16 = mybir.dt.bfloat16
FP8 = mybir.dt.float8e4
AF = mybir.ActivationFunctionType
ALU = mybir.AluOpType
DR = mybir.MatmulPerfMode.DoubleRow

KD = D // 128            # 8 k/m tiles over D
KE = KXP // 128          # 7 encoder k tiles
NS = 2                   # row halves of 512
NWARM = 8


def _build_program():
    nc = bacc.Bacc("TRN2", target_bir_lowering=False, debug=False)

    xt = nc.dram_tensor("xt", [KXP, R], BF16, kind="ExternalInput").ap()
    wc = nc.dram_tensor("wc", [KXP, D], BF16, kind="ExternalInput").ap()
    w8 = nc.dram_tensor("w8", [D, D], FP8, kind="ExternalInput").ap()
    bias = nc.dram_tensor("bias", [D], F32, kind="ExternalInput").ap()
    w2a = nc.dram_tensor("w2a", [D, 11], F32, kind="ExternalInput").ap()
    w1 = nc.dram_tensor("w1", [10], F32, kind="ExternalInput").ap()
    b2 = nc.dram_tensor("b2", [10], F32, kind="ExternalInput").ap()
    out = nc.dram_tensor("out", [R, 10], F32, kind="ExternalOutput").ap()

    with tile.TileContext(nc) as tc, ExitStack() as ctx:
        state = ctx.enter_context(tc.tile_pool(name="state", bufs=1))
        consts = ctx.enter_context(tc.tile_pool(name="consts", bufs=1))
        trans = ctx.enter_context(tc.tile_pool(name="trans", bufs=3))

        # persistent SBUF state, n-major so group ops are flat [128, 1024]
        drive = [state.tile([128, KD, 512], F32, name=f"dr{n}", tag=f"dr{n}")
                 for n in range(NS)]
        u_sb = [state.tile([128, KD, 512], F32R, name=f"u{n}", tag=f"u{n}")
                for n in range(NS)]
        u8_sb = [state.tile([128, KD, 512], FP8, name=f"u8{n}", tag=f"u8{n}")
                 for n in range(NS)]
        sq_sb = [state.tile([128, KD, 512], F32R, name=f"sq{n}", tag=f"sq{n}")
                 for n in range(NS)]
        w8_sb = state.tile([128, KD, D], FP8, name="w8sb")
        wc_sb = state.tile([128, KE, D], BF16, name="wcsb")
        xt_sb = state.tile([128, KE, R], BF16, name="xtsb")

        # input DMAs: everything on the sync queue (Act queue stays clear for
        # compute; gpsimd SWDGE is reserved for the loop cast-DMAs).
        bias_sb = consts.tile([128, KD], F32)
        nc.sync.dma_start(out=bias_sb, in_=bias.rearrange("(m p) -> p m", p=128))
        xt_r = xt.rearrange("(k p) r -> p k r", p=128)
        wc_r = wc.rearrange("(k p) c -> p k c", p=128)
        for k in range(KE):
            nc.sync.dma_start(out=xt_sb[:, k, :], in_=xt_r[:, k, :])
            nc.sync.dma_start(out=wc_sb[:, k, :], in_=wc_r[:, k, :])
        nc.sync.dma_start(out=w8_sb, in_=w8.rearrange("(k p) c -> p k c", p=128))

        ident = consts.tile([128, 128], F32)
        make_identity(nc, ident)
        w2a_sb = consts.tile([128, KD, 11], F32R)
        nc.sync.dma_start(out=w2a_sb,
                          in_=w2a.rearrange("(k p) o -> p k o", p=128).bitcast(F32R))
        ones_f32 = consts.tile([128, 1], F32)
        nc.vector.memset(ones_f32, 1.0)
        ones_sb = consts.tile([128, 1], F32R)
        nc.scalar.copy(ones_sb, ones_f32)
        eps_sb = consts.tile([128, 1], F32)
        nc.vector.memset(eps_sb, EPS)
        w1_bc = consts.tile([128, 10], F32)
        nc.sync.dma_start(out=w1_bc, in_=bass.AP(tensor=w1.tensor, offset=w1.offset,
                                                 ap=[[0, 128]] + list(w1.ap)))
        b2_bc = consts.tile([128, 10], F32)
        nc.sync.dma_start(out=b2_bc, in_=bass.AP(tensor=b2.tensor, offset=b2.offset,
                                                 ap=[[0, 128]] + list(b2.ap)))

        with ExitStack() as mmctx:
            encctx = ExitStack()
            psum = encctx.enter_context(
                tc.tile_pool(name="mm", bufs=4, space="PSUM"))

            # PE warmup: dependency-free matmuls at t~0 keep the HAM clock
            # gate at 8/8 while the input DMAs are in flight.
            warm_src = consts.tile([128, 256], F32)
            nc.vector.memset(warm_src, 0.01)
            warm_bf = consts.tile([128, 128], BF16)
            nc.vector.memset(warm_bf, 0.01)
            warm_sb = consts.tile([128, 1], F32)
            for w in range(NWARM):
                wp = psum.tile([128, 512], F32, name=f"warm{w}", tag="mm")
                nc.tensor.matmul(wp[:, :256], lhsT=warm_src[:, :128], rhs=warm_src,
                                 start=True, stop=True)
                if w == NWARM - 1:
                    nc.vector.tensor_copy(warm_sb, wp[:, :1])  # keep-alive

            # paced warmups: one matmul per landed xt chunk keeps the HAM
            # window busy across the input-DMA wait.
            for k in range(KE):
                wp = psum.tile([128, 512], F32, name=f"wx{k}", tag="mm")
                nc.tensor.matmul(wp[:, :256], lhsT=warm_bf,
                                 rhs=xt_sb[:, k, :256],
                                 start=True, stop=True)

            # ---------------- encoder: drive = (x @ W_c + bias).T ----------
            for n in range(NS):
                sl = slice(n * 512, (n + 1) * 512)
                for m in range(KD):
                    ps = psum.tile([128, 512], F32, name=f"eps{n}_{m}", tag="mm")
                    for k in range(KE):
                        nc.tensor.matmul(
                            ps, lhsT=wc_sb[:, k, m * 128:(m + 1) * 128],
                            rhs=xt_sb[:, k, sl],
                            start=(k == 0), stop=(k == KE - 1))
                    nc.scalar.activation(drive[n][:, m, :], ps, AF.Identity,
                                         bias=bias_sb[:, m:m + 1], scale=1.0)

            # ---------------- step 0: u1 = tanh(drive_in) ------------------
            for n in range(NS):
                for g in range(KD // 2):
                    ug = u_sb[n][:, 2 * g:2 * g + 2, :].opt()
                    nc.scalar.activation(ug, drive[n][:, 2 * g:2 * g + 2, :].opt(),
                                         AF.Tanh)
                    nc.gpsimd.dma_start(out=u8_sb[n][:, 2 * g:2 * g + 2, :],
                                        in_=u_sb[n][:, 2 * g:2 * g + 2, :].bitcast(F32))

            # ---------------- Euler loop, steps 1..9 -----------------------
            encctx.close()
            loopctx = ExitStack()
            psum2 = loopctx.enter_context(
                tc.tile_pool(name="mm2", bufs=3, space="PSUM"))
            wmp = loopctx.enter_context(
                tc.tile_pool(name="wm", bufs=1, space="PSUM"))
            wm_tile = wmp.tile([128, 512], F32, name="wmt")

            for s in range(1, STEPS):
                last = s == STEPS - 1
                for n in range(NS):
                    # dependency-free pacing matmul: holds the HAM gate open
                    # if the chain briefly stalls the PE.
                    nc.tensor.matmul(wm_tile[:, :256], lhsT=warm_src[:, :128],
                                     rhs=warm_src, start=True, stop=True)
                    for g in range(KD // 2):
                        ps = psum2.tile([128, 2, 512], F32, name=f"p{s}_{n}_{g}",
                                        tag="mm2")
                        for kp in range(KD // 2):
                            for mm in range(2):
                                mcol = (2 * g + mm) * 128
                                nc.tensor.matmul(
                                    ps[:, mm, :],
                                    lhsT=w8_sb[:, 2 * kp:2 * kp + 2,
                                               mcol:mcol + 128],
                                    rhs=u8_sb[n][:, 2 * kp:2 * kp + 2, :],
                                    perf_mode=DR,
                                    start=(kp == 0), stop=(kp == KD // 2 - 1))
                        d = trans.tile([128, 1024], F32, name=f"d{s}_{n}_{g}",
                                       tag="d")
                        nc.vector.scalar_tensor_tensor(
                            d, in0=ps.opt(), scalar=1.0 / S_W,
                            in1=drive[n][:, 2 * g:2 * g + 2, :].opt(),
                            op0=ALU.mult, op1=ALU.add)
                        t = trans.tile([128, 1024], F32, name=f"t{s}_{n}_{g}",
                                       tag="tau")
                        nc.scalar.activation(t, d, AF.Tanh)
                        ug = u_sb[n][:, 2 * g:2 * g + 2, :].opt()
                        nc.vector.scalar_tensor_tensor(
                            ug, in0=ug, scalar=1.0 - DT_STEP, in1=t,
                            op0=ALU.mult, op1=ALU.add)
                        if not last:
                            nc.gpsimd.dma_start(
                                out=u8_sb[n][:, 2 * g:2 * g + 2, :],
                                in_=u_sb[n][:, 2 * g:2 * g + 2, :].bitcast(F32))
                        else:
                            sg = sq_sb[n][:, 2 * g:2 * g + 2, :].opt()
                            nc.vector.tensor_tensor(sg, ug, ug, ALU.mult)

            # ---------------- tail: readout + LN stats ---------------------
            loopctx.close()
            psum = mmctx.enter_context(
                tc.tile_pool(name="mmt", bufs=2, space="PSUM"))
            tail = ctx.enter_context(tc.tile_pool(name="tail", bufs=1))
            s2_sb = tail.tile([1, R], F32)
            y_sb = tail.tile([11, R], F32)

            tp2ctx = ExitStack()
            tp2 = tp2ctx.enter_context(
                tc.tile_pool(name="tp2", bufs=4, space="PSUM"))
            for n in range(NS):
                sl = slice(n * 512, (n + 1) * 512)
                yp = psum.tile([11, 512], F32, name=f"yp{n}", tag="mm")
                for k in range(KD):
                    nc.tensor.matmul(yp, lhsT=w2a_sb[:, k, :],
                                     rhs=u_sb[n][:, k, :],
                                     start=(k == 0), stop=(k == KD - 1))
                nc.scalar.copy(y_sb[:, sl], yp)
                s2 = psum.tile([1, 512], F32, name=f"s2p{n}", tag="mm")
                for k in range(KD):
                    nc.tensor.matmul(s2, lhsT=ones_sb, rhs=sq_sb[n][:, k, :],
                                     start=(k == 0), stop=(k == KD - 1))
                nc.scalar.copy(s2_sb[:, sl], s2)

                for rt in range(n * 4, (n + 1) * 4):
                    rsl = slice(rt * 128, (rt + 1) * 128)
                    yn = tp2.tile([128, 11], F32, name=f"yn{rt}", tag="st")
                    nc.tensor.transpose(yn, y_sb[:, rsl], ident[:11, :11])
                    p2 = tp2.tile([128, 1], F32, name=f"p2_{rt}", tag="st")
                    nc.tensor.transpose(p2, s2_sb[:, rsl], ident[:1, :1])
                    mu_n = tail.tile([128, 1], F32, name=f"mu{rt}", tag="mu", bufs=2)
                    nc.scalar.mul(mu_n, yn[:, 10:11], -DT_STEP / D)   # -mean(h)
                    ex2 = tail.tile([128, 1], F32, name=f"ex2_{rt}", tag="ex2", bufs=2)
                    nc.scalar.mul(ex2, p2, DT_STEP * DT_STEP / D)     # E[h^2]
                    var = tail.tile([128, 1], F32, name=f"var{rt}", tag="var", bufs=2)
                    nc.vector.scalar_tensor_tensor(var, in0=mu_n, scalar=-1.0,
                                                   op0=ALU.mult, in1=mu_n,
                                                   op1=ALU.mult)
                    nc.vector.tensor_add(var, var, ex2)
                    sd = tail.tile([128, 1], F32, name=f"sd{rt}", tag="sd", bufs=2)
                    nc.scalar.activation(sd, var, AF.Sqrt, bias=eps_sb, scale=1.0)
                    inv = tail.tile([128, 1], F32, name=f"inv{rt}", tag="inv", bufs=2)
                    nc.vector.reciprocal(inv, sd)
                    qn = tail.tile([128, 1], F32, name=f"qn{rt}", tag="qn", bufs=2)
                    nc.vector.tensor_mul(qn, mu_n, inv)               # -mu*inv
                    t1 = tail.tile([128, 10], F32, name=f"t1_{rt}", tag="t1", bufs=2)
                    nc.vector.tensor_scalar_mul(t1, yn[:, 0:10], inv)
                    t2 = tail.tile([128, 10], F32, name=f"t2_{rt}", tag="t2", bufs=2)
                    nc.vector.scalar_tensor_tensor(t2, in0=w1_bc, scalar=qn,
                                                   in1=t1, op0=ALU.mult, op1=ALU.add)
                    o = tail.tile([128, 10], F32, name=f"o{rt}", tag="o", bufs=2)
                    nc.vector.tensor_add(o, t2, b2_bc)
                    nc.sync.dma_start(out=out[rsl, :], in_=o)
            tp2ctx.close()

    nc.compile()
    return nc


_NC_CACHE = None


def _get_program():
    global _NC_CACHE
    if _NC_CACHE is None:
        _NC_CACHE = _build_program()
    return _NC_CACHE


def _prepare_in_maps(inputs):
    x = np.asarray(inputs["x"], dtype=np.float32)
    w_enc = np.asarray(inputs["W_enc"], dtype=np.float32)
    w_res = np.asarray(inputs["W_res"], dtype=np.float32)
    w_in = np.asarray(inputs["W_in"], dtype=np.float32)
    bias = np.asarray(inputs["bias"], dtype=np.float32)
    ln_g = np.asarray(inputs["ln_g"], dtype=np.float32)
    ln_b = np.asarray(inputs["ln_b"], dtype=np.float32)
    w_out = np.asarray(inputs["W_out"], dtype=np.float32)
    b_out = np.asarray(inputs["b_out"], dtype=np.float32)

    w_c = (w_enc.T.astype(np.float64) @ w_in.astype(np.float64)).astype(np.float32)
    wc_pad = np.zeros((KXP, D), np.float32)
    wc_pad[:KX] = w_c
    w2 = w_out * ln_g[None, :]                       # [10, D]
    w8 = np.asarray(DT_STEP * S_W * w_res, dtype=ml_dtypes.float8_e4m3fn)
    w2a = np.zeros((D, 11), np.float32)
    w2a[:, :10] = DT_STEP * w2.T                     # readout: gives W2 @ h.T
    w2a[:, 10] = 1.0                                 # S1 column: sum_D u
    w1v = w2.sum(axis=1).astype(np.float32)
    b2v = (w_out.astype(np.float64) @ ln_b.astype(np.float64)
           + b_out.astype(np.float64)).astype(np.float32)

    shared = {
        "wc": np.ascontiguousarray(wc_pad.astype(ml_dtypes.bfloat16)),
        "w8": np.ascontiguousarray(w8),
        "bias": np.ascontiguousarray(bias),
        "w2a": np.ascontiguousarray(w2a),
        "w1": np.ascontiguousarray(w1v),
        "b2": np.ascontiguousarray(b2v),
    }
    in_maps = []
    for c in range(N_CORES):
        m = dict(shared)
        xt = np.zeros((KXP, R), np.float32)
        xt[:KX] = x[c * R:(c + 1) * R, :].T
        m["xt"] = np.ascontiguousarray(xt.astype(ml_dtypes.bfloat16))
        in_maps.append(m)
    return in_maps


def run(inputs, trace=False, tmpdir=None):
    """Run on 8 NeuronCores; returns (out [8192,10], BassKernelResults)."""
    nc = _get_program()
    in_maps = _prepare_in_maps(inputs)
    res = bass_utils.run_bass_kernel_spmd(
        nc, in_maps, core_ids=list(range(N_CORES)), trace=trace, tmpdir=tmpdir)
    outs = [np.asarray(r["out"]) for r in res.results]
    return np.concatenate(outs, axis=0), res


def kernel(**inputs):
    out, _ = run(inputs, trace=False)
    return out


# revision 7
# speedup vs baseline: 3.0483x; 2.4410x over previous
"""Trainium2 Bass kernel for nn_ChimeraNet (encoder -> 10-step Euler RNN -> LN -> readout).

Data-parallel over 8 NeuronCores: each core gets 1024 rows of the batch and a
replicated set of (host-prefolded) weights.

Math (per core, R=1024 rows, D=1024), with the state scaled u = h/0.2 so the
update is one fused op (0.2 folded into W_res and the readout on the host):
    drive_in = x @ W_c + bias          with W_c = W_enc.T @ W_in (host-folded)
    u' = 0.8 u + tanh(u @ (0.2 W_res) + drive_in),  u_0 = 0, 10 steps
    out = inv*(h @ W2.T) + (-mu*inv)*w1 + b2       (LayerNorm folded, h = 0.2 u)

fp8 fast path: the 9 recurrence matmuls run as fp8e4m3 DoubleRow (2x PE rate,
K=256 per instruction).  The matmul operand is u8 = fp8(u); W_res is scaled by
S=128 before fp8 quantization to clear the subnormal range, and the 1/S is
applied in the drive STT.  The state u is kept in bf16 (updated from the
unquantized bf16 tanh, which roughly halves the error vs fp8-tau feedback),
and all dtype conversions run on cast-DMAs (compute-engine casts take a
microcoded slow path).  Per step per (m-pair, n-half) group:
    PE   : 8 DoubleRow matmuls -> 2-bank PSUM tile      (u8 @ W8)
    DVE  : d = P*(1/S) + drive_in      (STT, flat [128,1024] APs)
    Act  : tau = tanh(d)               (f32->f32)
    DMA  : tau_b = bf16(tau); after the u STT, u8 = fp8(u)
    DVE  : u = 0.8 u + tau_b           (bf16, 2x DVE mode)
Encoder runs in bf16 (x pre-transposed/padded to [896, R] on the host - a
layout-only transform); LN stats via PE ones/readout-column tricks as before.
"""

import os
import sys

import numpy as np

try:
    import concourse.bass as bass  # noqa: F401
except ImportError:  # pragma: no cover - fresh grading env without PYTHONPATH
    for p in ("/root/.axon_site", "/root/.axon_site/_ro/trn_rl_repo",
              "/root/.axon_site/_ro/pypackages", "/opt/trn_rl_repo"):
        if os.path.isdir(p) and p not in sys.path:
            sys.path.append(p)
    import concourse.bass as bass

from contextlib import ExitStack

import ml_dtypes
import concourse.tile as tile
from concourse import bacc, bass_utils, mybir
from concourse.masks import make_identity

N_CORES = 8
B = 8192
R = B // N_CORES        # rows per core
D = 1024                # latent dim
KX = 784                # encoder input dim
KXP = 896               # padded to 7*128
DT_STEP = 0.2
STEPS = 10
EPS = 1e-5
S_W = 128.0             # fp8 scale on W_res

F32 = mybir.dt.float32
F32R = mybir.dt.float32r
BF16 = mybir.dt.bfloat16
FP8 = mybir.dt.float8e4
AF = mybir.ActivationFunctionType
ALU = mybir.AluOpType
DR = mybir.MatmulPerfMode.DoubleRow

KD = D // 128            # 8 k/m tiles over D
KE = KXP // 128          # 7 encoder k tiles
NS = 2                   # row halves of 512
NWARM = 8


def _build_program():
    nc = bacc.Bacc("TRN2", target_bir_lowering=False, debug=False)

    xt = nc.dram_tensor("xt", [KXP, R], BF16, kind="ExternalInput").ap()
    wc = nc.dram_tensor("wc", [KXP, D], BF16, kind="ExternalInput").ap()
    w8 = nc.dram_tensor("w8", [D, D], FP8, kind="ExternalInput").ap()
    bias = nc.dram_tensor("bias", [D], F32, kind="ExternalInput").ap()
    w2a = nc.dram_tensor("w2a", [D, 11], F32, kind="ExternalInput").ap()
    w1 = nc.dram_tensor("w1", [10], F32, kind="ExternalInput").ap()
    b2 = nc.dram_tensor("b2", [10], F32, kind="ExternalInput").ap()
    out = nc.dram_tensor("out", [R, 10], F32, kind="ExternalOutput").ap()

    with tile.TileContext(nc) as tc, ExitStack() as ctx:
        state = ctx.enter_context(tc.tile_pool(name="state", bufs=1))
        consts = ctx.enter_context(tc.tile_pool(name="consts", bufs=1))
        trans = ctx.enter_context(tc.tile_pool(name="trans", bufs=4))

        # persistent SBUF state, n-major so group ops are flat [128, 1024]
        drive = [state.tile([128, KD, 512], F32, name=f"dr{n}", tag=f"dr{n}")
                 for n in range(NS)]
        u_sb = [state.tile([128, KD, 512], F32R, name=f"u{n}", tag=f"u{n}")
                for n in range(NS)]
        u8_sb = [state.tile([128, KD, 512], FP8, name=f"u8{n}", tag=f"u8{n}")
                 for n in range(NS)]
        sq_sb = [state.tile([128, KD, 512], F32R, name=f"sq{n}", tag=f"sq{n}")
                 for n in range(NS)]
        w8_sb = state.tile([128, KD, D], FP8, name="w8sb")
        wc_sb = state.tile([128, KE, D], BF16, name="wcsb")
        xt_sb = state.tile([128, KE, R], BF16, name="xtsb")

        # input DMAs: everything on the sync queue (Act queue stays clear for
        # compute; gpsimd SWDGE is reserved for the loop cast-DMAs).
        bias_sb = consts.tile([128, KD], F32)
        nc.sync.dma_start(out=bias_sb, in_=bias.rearrange("(m p) -> p m", p=128))
        xt_r = xt.rearrange("(k p) r -> p k r", p=128)
        wc_r = wc.rearrange("(k p) c -> p k c", p=128)
        for k in range(KE):
            nc.sync.dma_start(out=xt_sb[:, k, :], in_=xt_r[:, k, :])
            nc.sync.dma_start(out=wc_sb[:, k, :], in_=wc_r[:, k, :])
        nc.sync.dma_start(out=w8_sb, in_=w8.rearrange("(k p) c -> p k c", p=128))

        ident = consts.tile([128, 128], F32)
        make_identity(nc, ident)
        w2a_sb = consts.tile([128, KD, 11], F32R)
        nc.sync.dma_start(out=w2a_sb,
                          in_=w2a.rearrange("(k p) o -> p k o", p=128).bitcast(F32R))
        ones_f32 = consts.tile([128, 1], F32)
        nc.vector.memset(ones_f32, 1.0)
        ones_sb = consts.tile([128, 1], F32R)
        nc.scalar.copy(ones_sb, ones_f32)
        eps_sb = consts.tile([128, 1], F32)
        nc.vector.memset(eps_sb, EPS)
        w1_bc = consts.tile([128, 10], F32)
        nc.sync.dma_start(out=w1_bc, in_=bass.AP(tensor=w1.tensor, offset=w1.offset,
                                                 ap=[[0, 128]] + list(w1.ap)))
        b2_bc = consts.tile([128, 10], F32)
        nc.sync.dma_start(out=b2_bc, in_=bass.AP(tensor=b2.tensor, offset=b2.offset,
                                                 ap=[[0, 128]] + list(b2.ap)))

        with ExitStack() as mmctx:
            encctx = ExitStack()
            psum = encctx.enter_context(
                tc.tile_pool(name="mm", bufs=4, space="PSUM"))

            # PE warmup: dependency-free matmuls at t~0 keep the HAM clock
            # gate at 8/8 while the input DMAs are in flight.
            warm_src = consts.tile([128, 256], F32)
            nc.vector.memset(warm_src, 0.01)
            warm_bf = consts.tile([128, 128], BF16)
            nc.vector.memset(warm_bf, 0.01)
            warm_sb = consts.tile([128, 1], F32)
            for w in range(NWARM):
                wp = psum.tile([128, 512], F32, name=f"warm{w}", tag="mm")
                nc.tensor.matmul(wp[:, :256], lhsT=warm_src[:, :128], rhs=warm_src,
                                 start=True, stop=True)
                if w == NWARM - 1:
                    nc.vector.tensor_copy(warm_sb, wp[:, :1])  # keep-alive

            # paced warmups: one matmul per landed xt chunk keeps the HAM
            # window busy across the input-DMA wait.
            for k in range(KE):
                wp = psum.tile([128, 512], F32, name=f"wx{k}", tag="mm")
                nc.tensor.matmul(wp[:, :256], lhsT=warm_bf,
                                 rhs=xt_sb[:, k, :256],
                                 start=True, stop=True)

            # ---------------- encoder: drive = (x @ W_c + bias).T ----------
            for n in range(NS):
                sl = slice(n * 512, (n + 1) * 512)
                for m in range(KD):
                    ps = psum.tile([128, 512], F32, name=f"eps{n}_{m}", tag="mm")
                    for k in range(KE):
                        nc.tensor.matmul(
                            ps, lhsT=wc_sb[:, k, m * 128:(m + 1) * 128],
                            rhs=xt_sb[:, k, sl],
                            start=(k == 0), stop=(k == KE - 1))
                    nc.scalar.activation(drive[n][:, m, :], ps, AF.Identity,
                                         bias=bias_sb[:, m:m + 1], scale=1.0)

            # ---------------- step 0: u1 = tanh(drive_in) ------------------
            for n in range(NS):
                for g in range(KD // 2):
                    ug = u_sb[n][:, 2 * g:2 * g + 2, :].opt()
                    nc.scalar.activation(ug, drive[n][:, 2 * g:2 * g + 2, :].opt(),
                                         AF.Tanh)
                    nc.gpsimd.dma_start(out=u8_sb[n][:, 2 * g:2 * g + 2, :],
                                        in_=u_sb[n][:, 2 * g:2 * g + 2, :].bitcast(F32))

            # ---------------- Euler loop, steps 1..9 -----------------------
            encctx.close()
            loopctx = ExitStack()
            psum2 = loopctx.enter_context(
                tc.tile_pool(name="mm2", bufs=3, space="PSUM"))
            wmp = loopctx.enter_context(
                tc.tile_pool(name="wm", bufs=1, space="PSUM"))
            wm_tile = wmp.tile([128, 512], F32, name="wmt")

            for s in range(1, STEPS):
                last = s == STEPS - 1
                for n in range(NS):
                    # dependency-free pacing matmul: holds the HAM gate open
                    # if the chain briefly stalls the PE.
                    nc.tensor.matmul(wm_tile[:, :256], lhsT=warm_src[:, :128],
                                     rhs=warm_src, start=True, stop=True)
                    NG = KD // 2
                    pss, ds, ts = [], [], []
                    for g in range(NG):
                        ps = psum2.tile([128, 2, 512], F32, name=f"p{s}_{n}_{g}",
                                        tag="mm2")
                        pss.append(ps)
                        for kp in range(KD // 2):
                            for mm in range(2):
                                mcol = (2 * g + mm) * 128
                                nc.tensor.matmul(
                                    ps[:, mm, :],
                                    lhsT=w8_sb[:, 2 * kp:2 * kp + 2,
                                               mcol:mcol + 128],
                                    rhs=u8_sb[n][:, 2 * kp:2 * kp + 2, :],
                                    perf_mode=DR,
                                    start=(kp == 0), stop=(kp == KD // 2 - 1))

                    # skewed emission: DVE order d0,d1,u0,d2,u1,d3,u2,u3 so the
                    # in-order DVE queue never stalls on a tanh; Act runs the
                    # tanhs back-to-back; casts trail the u-updates.
                    def emit_d(g):
                        d = trans.tile([128, 1024], F32, name=f"d{s}_{n}_{g}",
                                       tag="d")
                        ds.append(d)
                        nc.vector.scalar_tensor_tensor(
                            d, in0=pss[g].opt(), scalar=1.0 / S_W,
                            in1=drive[n][:, 2 * g:2 * g + 2, :].opt(),
                            op0=ALU.mult, op1=ALU.add)

                    def emit_t(g):
                        t = trans.tile([128, 1024], F32, name=f"t{s}_{n}_{g}",
                                       tag="tau")
                        ts.append(t)
                        nc.scalar.activation(t, ds[g], AF.Tanh)

                    def emit_u(g):
                        ug = u_sb[n][:, 2 * g:2 * g + 2, :].opt()
                        nc.vector.scalar_tensor_tensor(
                            ug, in0=ug, scalar=1.0 - DT_STEP, in1=ts[g],
                            op0=ALU.mult, op1=ALU.add)
                        if not last:
                            nc.gpsimd.dma_start(
                                out=u8_sb[n][:, 2 * g:2 * g + 2, :],
                                in_=u_sb[n][:, 2 * g:2 * g + 2, :].bitcast(F32))
                        else:
                            sg = sq_sb[n][:, 2 * g:2 * g + 2, :].opt()
                            nc.vector.tensor_tensor(
                                sg, u_sb[n][:, 2 * g:2 * g + 2, :].opt(),
                                u_sb[n][:, 2 * g:2 * g + 2, :].opt(), ALU.mult)

                    emit_d(0); emit_t(0)
                    emit_d(1); emit_t(1)
                    emit_u(0)
                    emit_d(2); emit_t(2)
                    emit_u(1)
                    emit_d(3); emit_t(3)
                    emit_u(2)
                    emit_u(3)

            # ---------------- tail: readout + LN stats ---------------------
            loopctx.close()
            psum = mmctx.enter_context(
                tc.tile_pool(name="mmt", bufs=2, space="PSUM"))
            tail = ctx.enter_context(tc.tile_pool(name="tail", bufs=1))
            s2_sb = tail.tile([1, R], F32)
            y_sb = tail.tile([11, R], F32)

            tp2ctx = ExitStack()
            tp2 = tp2ctx.enter_context(
                tc.tile_pool(name="tp2", bufs=4, space="PSUM"))
            for n in range(NS):
                sl = slice(n * 512, (n + 1) * 512)
                yp = psum.tile([11, 512], F32, name=f"yp{n}", tag="mm")
                for k in range(KD):
                    nc.tensor.matmul(yp, lhsT=w2a_sb[:, k, :],
                                     rhs=u_sb[n][:, k, :],
                                     start=(k == 0), stop=(k == KD - 1))
                nc.scalar.copy(y_sb[:, sl], yp)
                s2 = psum.tile([1, 512], F32, name=f"s2p{n}", tag="mm")
                for k in range(KD):
                    nc.tensor.matmul(s2, lhsT=ones_sb, rhs=sq_sb[n][:, k, :],
                                     start=(k == 0), stop=(k == KD - 1))
                nc.scalar.copy(s2_sb[:, sl], s2)

                for rt in range(n * 4, (n + 1) * 4):
                    rsl = slice(rt * 128, (rt + 1) * 128)
                    yn = tp2.tile([128, 11], F32, name=f"yn{rt}", tag="st")
                    nc.tensor.transpose(yn, y_sb[:, rsl], ident[:11, :11])
                    p2 = tp2.tile([128, 1], F32, name=f"p2_{rt}", tag="st")
                    nc.tensor.transpose(p2, s2_sb[:, rsl], ident[:1, :1])
                    mu_n = tail.tile([128, 1], F32, name=f"mu{rt}", tag="mu", bufs=2)
                    nc.scalar.mul(mu_n, yn[:, 10:11], -DT_STEP / D)   # -mean(h)
                    ex2 = tail.tile([128, 1], F32, name=f"ex2_{rt}", tag="ex2", bufs=2)
                    nc.scalar.mul(ex2, p2, DT_STEP * DT_STEP / D)     # E[h^2]
                    var = tail.tile([128, 1], F32, name=f"var{rt}", tag="var", bufs=2)
                    nc.vector.scalar_tensor_tensor(var, in0=mu_n, scalar=-1.0,
                                                   op0=ALU.mult, in1=mu_n,
                                                   op1=ALU.mult)
                    nc.vector.tensor_add(var, var, ex2)
                    sd = tail.tile([128, 1], F32, name=f"sd{rt}", tag="sd", bufs=2)
                    nc.scalar.activation(sd, var, AF.Sqrt, bias=eps_sb, scale=1.0)
                    inv = tail.tile([128, 1], F32, name=f"inv{rt}", tag="inv", bufs=2)
                    nc.vector.reciprocal(inv, sd)
                    qn = tail.tile([128, 1], F32, name=f"qn{rt}", tag="qn", bufs=2)
                    nc.vector.tensor_mul(qn, mu_n, inv)               # -mu*inv
                    t1 = tail.tile([128, 10], F32, name=f"t1_{rt}", tag="t1", bufs=2)
                    nc.vector.tensor_scalar_mul(t1, yn[:, 0:10], inv)
                    t2 = tail.tile([128, 10], F32, name=f"t2_{rt}", tag="t2", bufs=2)
                    nc.vector.scalar_tensor_tensor(t2, in0=w1_bc, scalar=qn,
                                                   in1=t1, op0=ALU.mult, op1=ALU.add)
                    o = tail.tile([128, 10], F32, name=f"o{rt}", tag="o", bufs=2)
                    nc.vector.tensor_add(o, t2, b2_bc)
                    nc.sync.dma_start(out=out[rsl, :], in_=o)
            tp2ctx.close()

    nc.compile()
    return nc


_NC_CACHE = None


def _get_program():
    global _NC_CACHE
    if _NC_CACHE is None:
        _NC_CACHE = _build_program()
    return _NC_CACHE


def _prepare_in_maps(inputs):
    x = np.asarray(inputs["x"], dtype=np.float32)
    w_enc = np.asarray(inputs["W_enc"], dtype=np.float32)
    w_res = np.asarray(inputs["W_res"], dtype=np.float32)
    w_in = np.asarray(inputs["W_in"], dtype=np.float32)
    bias = np.asarray(inputs["bias"], dtype=np.float32)
    ln_g = np.asarray(inputs["ln_g"], dtype=np.float32)
    ln_b = np.asarray(inputs["ln_b"], dtype=np.float32)
    w_out = np.asarray(inputs["W_out"], dtype=np.float32)
    b_out = np.asarray(inputs["b_out"], dtype=np.float32)

    w_c = (w_enc.T.astype(np.float64) @ w_in.astype(np.float64)).astype(np.float32)
    wc_pad = np.zeros((KXP, D), np.float32)
    wc_pad[:KX] = w_c
    w2 = w_out * ln_g[None, :]                       # [10, D]
    w8 = np.asarray(DT_STEP * S_W * w_res, dtype=ml_dtypes.float8_e4m3fn)
    w2a = np.zeros((D, 11), np.float32)
    w2a[:, :10] = DT_STEP * w2.T                     # readout: gives W2 @ h.T
    w2a[:, 10] = 1.0                                 # S1 column: sum_D u
    w1v = w2.sum(axis=1).astype(np.float32)
    b2v = (w_out.astype(np.float64) @ ln_b.astype(np.float64)
           + b_out.astype(np.float64)).astype(np.float32)

    shared = {
        "wc": np.ascontiguousarray(wc_pad.astype(ml_dtypes.bfloat16)),
        "w8": np.ascontiguousarray(w8),
        "bias": np.ascontiguousarray(bias),
        "w2a": np.ascontiguousarray(w2a),
        "w1": np.ascontiguousarray(w1v),
        "b2": np.ascontiguousarray(b2v),
    }
    in_maps = []
    for c in range(N_CORES):
        m = dict(shared)
        xt = np.zeros((KXP, R), np.float32)
        xt[:KX] = x[c * R:(c + 1) * R, :].T
        m["xt"] = np.ascontiguousarray(xt.astype(ml_dtypes.bfloat16))
        in_maps.append(m)
    return in_maps


def run(inputs, trace=False, tmpdir=None):
    """Run on 8 NeuronCores; returns (out [8192,10], BassKernelResults)."""
    nc = _get_program()
    in_maps = _prepare_in_maps(inputs)
    res = bass_utils.run_bass_kernel_spmd(
        nc, in_maps, core_ids=list(range(N_CORES)), trace=trace, tmpdir=tmpdir)
    outs = [np.asarray(r["out"]) for r in res.results]
    return np.concatenate(outs, axis=0), res


def kernel(**inputs):
    out, _ = run(inputs, trace=False)
    return out
